# revision 31
# baseline (speedup 1.0000x reference)
"""GPT-2 attention block (B=2, S=2048, E=1024, H=16) on 8 TRN2 NeuronCores.

Sharding: 8-way tensor parallel over heads (2 heads/core). Host passes
x pre-transposed (xT [E,T] bf16), so the qkv projection consumes it
directly with no on-device transposes. Four chunked AllToAlls reshard
attention output from head-sharded to token-sharded (1024 tokens each),
overlapping collectives with attention compute; each core computes the
c_proj output for its 4x128-token slices with full contraction.

All matmuls run in bf16 (full-rate PE + FWL weight loads); softmax
logits accumulate to bf16 PSUM tiles (1 bank each) so the S-tile pool
is 4 deep. Emission interleaves QK(s) with PV(s-1) and qkv/proj filler
so the PE stays dense (HAM warm) while ACT streams the exps.

Per-core dataflow:
  qT = Wq^T xT (+bq)  [128, 4096] bf16     (DVE evac, per-partition bias)
  kT, vT likewise; V native layout via PE transpose of vT
  per 512-token q tile: S^T = K Q^T (bf16 PSUM); P^T = exp(S^T/8) on ACT
  O'^T[65,512] = [V|1]^T P^T  (row 64 = softmax denominators, unnormalized)
  4x AllToAll of O'^T+sums (bf16) -> token-sharded og
  og_n = og * broadcast(1/sums)  (approx reciprocal, 16 lanes)
  y = og_n @ Wp + bp -> out [512, 1024] f32
"""

import sys

if "/opt/trn_rl_repo" not in sys.path:
    sys.path.insert(0, "/opt/trn_rl_repo")

import ml_dtypes
import numpy as np

import concourse.bass as bass  # noqa: F401
import concourse.mybir as mybir
from concourse import bacc, tile
from concourse.bass_utils import run_bass_kernel_spmd
from concourse.masks import make_identity

F32 = mybir.dt.float32
BF16 = mybir.dt.bfloat16
AF = mybir.ActivationFunctionType

B, S, E, H = 2, 2048, 1024, 16
D = E // H            # 64
NC = 8                # cores
HPC = H // NC         # 2 heads per core
FPC = HPC * D         # 128 per-core q/k/v feature count
T = B * S             # 4096 tokens, batch-major
TC = T // NC          # 512 output tokens per core
NTT = T // 128        # 32 token tiles of 128
NST = T // 512        # 8 token supertiles of 512
NEC = E // 128        # 8 contraction chunks
KT_PER_B = S // 128   # 16 k tiles per batch
NSLOT = T // 512      # 8 attention slots of 512 q tokens
NA2A = 8              # chunked A2As, one per slot (512 tokens each)
ATOK = T // NA2A // NC  # 64 tokens per core per A2A


def build_nc():
    nc = bacc.Bacc("TRN2", target_bir_lowering=False, debug=False, num_devices=NC)

    xT_ext = nc.dram_tensor("xT", [E, T], BF16, kind="ExternalInput")
    wq_ext = nc.dram_tensor("wq", [E, FPC], BF16, kind="ExternalInput")
    wk_ext = nc.dram_tensor("wk", [E, FPC], BF16, kind="ExternalInput")
    wv_ext = nc.dram_tensor("wv", [E, FPC], BF16, kind="ExternalInput")
    wp_ext = nc.dram_tensor("wp", [E, E], BF16, kind="ExternalInput")
    bq_ext = nc.dram_tensor("bq", [FPC], F32, kind="ExternalInput")
    bk_ext = nc.dram_tensor("bk", [FPC], F32, kind="ExternalInput")
    bv_ext = nc.dram_tensor("bv", [FPC], F32, kind="ExternalInput")
    bp_ext = nc.dram_tensor("bp", [E], BF16, kind="ExternalInput")
    out_ext = nc.dram_tensor("out", [TC, E], F32, kind="ExternalOutput")

    # A2A bounce buffers: A2A m moves, for each dest core j, my (normalized)
    # oT columns for tokens [1024m + 128j, +128).
    o_loc = [nc.dram_tensor(f"o_loc{m}", [NC, FPC, ATOK], BF16) for m in range(NA2A)]
    o_gat = [nc.dram_tensor(f"o_gat{m}", [NC, FPC, ATOK], BF16) for m in range(NA2A)]

    with tile.TileContext(nc) as tc:
        with (
            tc.tile_pool(name="const", bufs=1) as cpool,
            tc.tile_pool(name="wqkv", bufs=1) as wpool,
            tc.tile_pool(name="persist", bufs=1) as apool,
            tc.tile_pool(name="xst", bufs=3) as xpool,
            tc.tile_pool(name="vT", bufs=2) as vtpool,
            tc.tile_pool(name="pT", bufs=34) as ppool,
            tc.tile_pool(name="og", bufs=2) as ogpool,
            tc.tile_pool(name="nrm", bufs=4) as npool,
            tc.tile_pool(name="ysb", bufs=2) as ypool,
            tc.tile_pool(name="ps_s", bufs=2, space="PSUM") as ps_s_pool,
            tc.tile_pool(name="ps_o", bufs=2, space="PSUM") as ps_o_pool,
            tc.tile_pool(name="ps_m", bufs=2, space="PSUM") as ps_m_pool,
        ):
            ident_f = cpool.tile([128, 128], F32)
            make_identity(nc, ident_f[:])
            ident = cpool.tile([128, 128], BF16)
            nc.vector.tensor_copy(ident[:], ident_f[:])
            ones_r = cpool.tile([1, 128], BF16)
            nc.vector.memset(ones_r[:], 1.0)
            bq_sb = cpool.tile([128, 1], F32)
            bk_sb = cpool.tile([128, 1], F32)
            bv_sb = cpool.tile([128, 1], F32)
            bp_sb = cpool.tile([1, E], BF16)
            nc.sync.dma_start(out=bq_sb[:], in_=bq_ext.ap().rearrange("(p a) -> p a", p=FPC))
            nc.sync.dma_start(out=bk_sb[:], in_=bk_ext.ap().rearrange("(p a) -> p a", p=FPC))
            nc.sync.dma_start(out=bv_sb[:], in_=bv_ext.ap().rearrange("(p a) -> p a", p=FPC))
            nc.sync.dma_start(out=bp_sb[:], in_=bp_ext.ap().rearrange("(a f) -> a f", a=1))

            wq_sb = wpool.tile([128, NEC, FPC], BF16)
            wk_sb = wpool.tile([128, NEC, FPC], BF16)
            wv_sb = wpool.tile([128, NEC, FPC], BF16)
            nc.sync.dma_start(out=wq_sb[:], in_=wq_ext.ap().rearrange("(j p) f -> p j f", p=128))
            nc.sync.dma_start(out=wk_sb[:], in_=wk_ext.ap().rearrange("(j p) f -> p j f", p=128))
            nc.sync.dma_start(out=wv_sb[:], in_=wv_ext.ap().rearrange("(j p) f -> p j f", p=128))
            wp_sb = apool.tile([128, NEC, E], BF16)

            qT = apool.tile([128, T], BF16)   # q features x all tokens
            kT = apool.tile([128, T], BF16)
            v_all = apool.tile([128, NTT, HPC, D + 1], BF16)  # [tok128, ktile, head, V|1]
            oT = apool.tile([128, T], BF16)   # attention out channels x tokens


            # ones column of v_all (softmax row-sum trick)
            nc.vector.memset(v_all[:, :, :, D : D + 1], 1.0)

            # ---------- phase building blocks ------------------------------
            def qkv_supertile(st):
                """qkv projection for 512 tokens using host-transposed xT."""
                x_t = xpool.tile([128, NEC, 512], BF16, tag="x")
                nc.sync.dma_start(
                    out=x_t[:],
                    in_=xT_ext[:, st * 512 : (st + 1) * 512].rearrange(
                        "(j p) t -> p j t", p=128
                    ),
                )
                vT_st = vtpool.tile([128, 512], BF16, tag="vt")
                for w_sb, b_sb, dst in (
                    (wq_sb, bq_sb, qT[:, st * 512 : (st + 1) * 512]),
                    (wk_sb, bk_sb, kT[:, st * 512 : (st + 1) * 512]),
                    (wv_sb, bv_sb, vT_st[:]),
                ):
                    ps = ps_m_pool.tile([128, 512], F32, tag="m")
                    for j in range(NEC):
                        nc.tensor.matmul(
                            ps[:],
                            w_sb[:, j, :],
                            x_t[:, j, :],
                            start=(j == 0),
                            stop=(j == NEC - 1),
                        )
                    nc.vector.tensor_scalar_add(dst, ps[:], b_sb[:])
                # V native layout via PE transpose of vT (transpose-mode
                # output dtype must match input: bf16, half a PSUM bank)
                ps_v = ps_m_pool.tile([128, 512], BF16, tag="m")
                for i in range(4):
                    nc.tensor.transpose(
                        ps_v[:, 128 * i : 128 * (i + 1)],
                        vT_st[:, 128 * i : 128 * (i + 1)],
                        ident[:],
                    )
                nc.vector.tensor_copy(
                    v_all[:, st * 4 : (st + 1) * 4, :, 0:D],
                    ps_v[:].rearrange("p (i h d) -> p i h d", i=4, h=HPC),
                )

            def qk_group(s, h, tt, pts):
                """S^T tile (2 k-tiles) for slot s head h + exp -> P^T."""
                b = s // 4
                q0 = s * 512
                hp = 64 * h
                ps_s = ps_s_pool.tile([128, 1024], F32, tag="s")
                for i in range(2):
                    kti = b * KT_PER_B + tt * 2 + i
                    nc.tensor.matmul(
                        ps_s[:, 512 * i : 512 * (i + 1)],
                        kT[hp : hp + 64, 128 * kti : 128 * (kti + 1)],
                        qT[hp : hp + 64, q0 : q0 + 512],
                        start=True,
                        stop=True,
                        tile_position=(64 * h, 0),
                    )
                pt = ppool.tile([128, 1024], BF16, tag="p")
                nc.scalar.activation(pt[:], ps_s[:], AF.Exp, scale=0.125)
                pts[(h, tt)] = pt

            def pv_chunk(s, h, c, pts, ps_o_box):
                """4 accumulating PV matmuls (k-tiles 4c..4c+3) of slot s."""
                b = s // 4
                if c == 0:
                    ps_o_box[h] = ps_o_pool.tile([128, 512], F32, tag="o", name="ps_o")
                ps_o = ps_o_box[h]
                for kk in range(4):
                    kt = 4 * c + kk
                    kti = b * KT_PER_B + kt
                    nc.tensor.matmul(
                        ps_o[0 : D + 1, :],
                        v_all[:, kti, h, :],
                        pts[(h, kt // 2)][:, 512 * (kt % 2) : 512 * (kt % 2 + 1)],
                        start=(kt == 0),
                        stop=(kt == KT_PER_B - 1),
                    )

            def pv_evac(s, h, ps_o_box):
                """Normalize by the softmax denominator (row D) and evac."""
                q0 = s * 512
                hp = 64 * h
                ps_o = ps_o_box[h]
                # stage sums to SBUF: the custom-DVE recip mis-addresses a
                # PSUM source with nonzero base partition
                sm = npool.tile([1, 512], F32, tag="sm")
                nc.vector.tensor_copy(sm[:], ps_o[D : D + 1, :])
                rc = npool.tile([1, 512], F32, tag="rc")
                nc.vector.reciprocal_approx_fast(rc[:], sm[:])
                bcs = npool.tile([128, 512], F32, tag="bc")
                nc.gpsimd.partition_broadcast(bcs[:], rc[:])
                nc.vector.tensor_mul(
                    oT[hp : hp + 64, q0 : q0 + 512], ps_o[0:D, :], bcs[0:D, :]
                )

            def a2a_start(m):
                t0 = m * (T // NA2A)
                for j in range(NC):
                    c0 = t0 + ATOK * j
                    nc.sync.dma_start(out=o_loc[m][j], in_=oT[:, c0 : c0 + ATOK])
                nc.gpsimd.collective_compute(
                    "AllToAll",
                    mybir.AluOpType.bypass,
                    replica_groups=[list(range(NC))],
                    ins=[o_loc[m].ap().opt()],
                    outs=[o_gat[m].ap().opt()],
                )

            def og_load(m, og, half):
                """Gather A2A m output (already normalized) into one half of
                a paired og tile (proj consumes two A2As = 128 tokens)."""
                for j in range(NC):
                    nc.sync.dma_start(
                        out=og[:, j, ATOK * half : ATOK * (half + 1)], in_=o_gat[m][j]
                    )

            def proj(m, og_n):
                """c_proj for my 128 tokens of A2A pair m."""
                for cb in range(2):
                    ps_y = ps_m_pool.tile([128, 512], F32, tag="m")
                    for j in range(NEC):
                        nc.tensor.matmul(
                            ps_y[:],
                            og_n[:, j, :],
                            wp_sb[:, j, 512 * cb : 512 * (cb + 1)],
                            start=(j == 0),
                            stop=False,
                        )
                    nc.tensor.matmul(
                        ps_y[:],
                        ones_r[:, 0:128],
                        bp_sb[:, 512 * cb : 512 * (cb + 1)],
                        start=False,
                        stop=True,
                    )
                    y_sb = ypool.tile([128, 512], F32, tag="y")
                    nc.vector.tensor_copy(y_sb[:], ps_y[:])
                    nc.sync.dma_start(
                        out=out_ext[128 * m : 128 * (m + 1), 512 * cb : 512 * (cb + 1)],
                        in_=y_sb[:],
                    )

            # ---------- emission schedule ----------------------------------
            # Slot s computes QK+exp for q tokens [512s, 512s+512) while the
            # PE drains PV of slot s-1 between QK groups. qkv supertiles are
            # woven in: st0 up front, st1-3 inside slot 0 right before the
            # QK groups that consume them, st4-7 (batch 1) as filler in
            # slots 1-3. proj blocks inject after g==3 where the ACT has
            # backlog to hide the PE detour; A2A m fires as soon as its two
            # source slots are evacuated.
            qkv_supertile(0)

            pts_prev = None
            ps_o_prev: dict = {}
            og_ns: dict = {}
            for s in range(NSLOT + 1):
                pts: dict = {}
                ps_o_box: dict = {}
                for g in range(8):
                    if s == 0 and g in (2, 4, 6):
                        qkv_supertile(g // 2)  # st 1,2,3 feed QK groups below
                    if s < NSLOT:
                        for h in range(HPC):
                            qk_group(s, h, g, pts)
                    if s > 0:
                        h, c = (0, g) if g < 4 else (1, g - 4)
                        pv_chunk(s - 1, h, c, pts_prev, ps_o_prev)
                        if g == 3:
                            pv_evac(s - 1, 0, ps_o_prev)
                        elif g == 7:
                            pv_evac(s - 1, 1, ps_o_prev)
                            a2a_start(s - 1)
                    if g == 3:
                        # mid-slot fillers: ACT has backlog here, so a PE
                        # detour doesn't starve it
                        if 1 <= s <= 2:
                            qkv_supertile(s + 3)  # st 4, 5
                        elif s == 5:
                            proj(0, og_ns[0])
                        elif s == 7:
                            proj(1, og_ns[1])
                    elif g == 5 and s == 3:
                        qkv_supertile(7)
                    elif g == 1 and s == 3:
                        qkv_supertile(6)
                # end-of-slot: og gathers for completed A2As
                if s == 0:
                    nc.sync.dma_start(
                        out=wp_sb[:],
                        in_=wp_ext.ap().rearrange("(j p) f -> p j f", p=128),
                    )
                elif s == 3:
                    og_ns[0] = ogpool.tile([128, NC, 2 * ATOK], BF16, tag="og", name="og0")
                    og_load(0, og_ns[0], 0)
                elif s == 4:
                    og_load(1, og_ns[0], 1)
                    og_ns[1] = ogpool.tile([128, NC, 2 * ATOK], BF16, tag="og", name="og1")
                    og_load(2, og_ns[1], 0)
                elif s == 5:
                    og_load(3, og_ns[1], 1)
                elif s == 6:
                    og_ns[2] = ogpool.tile([128, NC, 2 * ATOK], BF16, tag="og", name="og2")
                    og_load(4, og_ns[2], 0)
                elif s == 7:
                    og_load(5, og_ns[2], 1)
                elif s == 8:
                    og_ns[3] = ogpool.tile([128, NC, 2 * ATOK], BF16, tag="og", name="og3")
                    og_load(6, og_ns[3], 0)
                    og_load(7, og_ns[3], 1)
                    proj(2, og_ns[2])
                    proj(3, og_ns[3])
                pts_prev = pts
                ps_o_prev = ps_o_box

    nc.compile()
    return nc


_NC_CACHE = None


def _get_nc():
    global _NC_CACHE
    if _NC_CACHE is None:
        _NC_CACHE = build_nc()
    return _NC_CACHE


def kernel(
    hidden_states: np.ndarray,
    c_attn_w: np.ndarray,
    c_attn_b: np.ndarray,
    c_proj_w: np.ndarray,
    c_proj_b: np.ndarray,
    _want_results_obj: bool = False,
    **_unused,
) -> np.ndarray:
    BF = ml_dtypes.bfloat16
    x = np.asarray(hidden_states, dtype=np.float32).reshape(T, E)
    xT = np.ascontiguousarray(x.T).astype(BF)
    w = np.asarray(c_attn_w, dtype=np.float32)
    battn = np.asarray(c_attn_b, dtype=np.float32)
    wp = np.ascontiguousarray(np.asarray(c_proj_w, dtype=np.float32)).astype(BF)
    bp = np.asarray(c_proj_b, dtype=np.float32).astype(BF)

    in_maps = []
    for c in range(NC):
        f0 = FPC * c
        in_maps.append(
            {
                "xT": xT,
                "wq": np.ascontiguousarray(w[:, f0 : f0 + FPC].astype(BF)),
                "wk": np.ascontiguousarray(w[:, E + f0 : E + f0 + FPC].astype(BF)),
                "wv": np.ascontiguousarray(w[:, 2 * E + f0 : 2 * E + f0 + FPC].astype(BF)),
                "wp": wp,
                "bq": np.ascontiguousarray(battn[f0 : f0 + FPC]),
                "bk": np.ascontiguousarray(battn[E + f0 : E + f0 + FPC]),
                "bv": np.ascontiguousarray(battn[2 * E + f0 : 2 * E + f0 + FPC]),
                "bp": bp,
            }
        )

    nc = _get_nc()
    res = run_bass_kernel_spmd(nc, in_maps, core_ids=list(range(NC)))
    y = np.empty((T, E), dtype=np.float32)
    for c in range(NC):
        for m in range(NA2A):
            t0 = m * (T // NA2A) + ATOK * c
            y[t0 : t0 + ATOK] = res.results[c]["out"][ATOK * m : ATOK * (m + 1)]
    # (out row u of core c holds global token 512*(u//64) + 64*c + u%64)
    out = y.reshape(B, S, E)
    if _want_results_obj:
        return out, res
    return out


# revision 39
# speedup vs baseline: 1.1044x; 1.1044x over previous
"""GPT-2 attention block (B=2, S=2048, E=1024, H=16) on 8 TRN2 NeuronCores.

Sharding: 8-way tensor parallel over heads (2 heads/core). Host passes
x pre-transposed (xT [E,T] bf16), so the qkv projection consumes it
directly with no on-device transposes. Four chunked AllToAlls reshard
attention output from head-sharded to token-sharded (1024 tokens each),
overlapping collectives with attention compute; each core computes the
c_proj output for its 4x128-token slices with full contraction.

All matmuls run in bf16 (full-rate PE + FWL weight loads); softmax
logits accumulate to bf16 PSUM tiles (1 bank each) so the S-tile pool
is 4 deep. Emission interleaves QK(s) with PV(s-1) and qkv/proj filler
so the PE stays dense (HAM warm) while ACT streams the exps.

Per-core dataflow:
  qT = Wq^T xT (+bq)  [128, 4096] bf16     (DVE evac, per-partition bias)
  kT, vT likewise; V native layout via PE transpose of vT
  per 512-token q tile: S^T = K Q^T (bf16 PSUM); P^T = exp(S^T/8) on ACT
  O'^T[65,512] = [V|1]^T P^T  (row 64 = softmax denominators, unnormalized)
  4x AllToAll of O'^T+sums (bf16) -> token-sharded og
  og_n = og * broadcast(1/sums)  (approx reciprocal, 16 lanes)
  y = og_n @ Wp + bp -> out [512, 1024] f32
"""

import sys

if "/opt/trn_rl_repo" not in sys.path:
    sys.path.insert(0, "/opt/trn_rl_repo")

import ml_dtypes
import numpy as np

import concourse.bass as bass  # noqa: F401
import concourse.mybir as mybir
from concourse import bacc, tile
from concourse.bass_utils import run_bass_kernel_spmd
from concourse.masks import make_identity

F32 = mybir.dt.float32
BF16 = mybir.dt.bfloat16
AF = mybir.ActivationFunctionType

B, S, E, H = 2, 2048, 1024, 16
D = E // H            # 64
NC = 8                # cores
HPC = H // NC         # 2 heads per core
FPC = HPC * D         # 128 per-core q/k/v feature count
T = B * S             # 4096 tokens, batch-major
TC = T // NC          # 512 output tokens per core
NTT = T // 128        # 32 token tiles of 128
NST = T // 512        # 8 token supertiles of 512
NEC = E // 128        # 8 contraction chunks
KT_PER_B = S // 128   # 16 k tiles per batch
NSLOT = T // 512      # 8 attention slots of 512 q tokens
ATOK = 64             # tokens per core per slot-chunk
# A2A groups of slots: early ones pair 2 slots (fewer collectives, the
# ~13us-floor cc stream must not backlog); the last two are single-slot
# so the tail collective after the final evac is minimal.
A2A_GROUPS = [(0, 1), (2, 3), (4, 5), (6,), (7,)]
SLOT_GROUP = {s: (gi, idx) for gi, g in enumerate(A2A_GROUPS) for idx, s in enumerate(g)}


def build_nc():
    nc = bacc.Bacc("TRN2", target_bir_lowering=False, debug=False, num_devices=NC)

    xT_ext = nc.dram_tensor("xT", [E, T], BF16, kind="ExternalInput")
    wq_ext = nc.dram_tensor("wq", [E, FPC], BF16, kind="ExternalInput")
    wk_ext = nc.dram_tensor("wk", [E, FPC], BF16, kind="ExternalInput")
    wv_ext = nc.dram_tensor("wv", [E, FPC], BF16, kind="ExternalInput")
    wp_ext = nc.dram_tensor("wp", [E, E], BF16, kind="ExternalInput")
    bq_ext = nc.dram_tensor("bq", [FPC], F32, kind="ExternalInput")
    bk_ext = nc.dram_tensor("bk", [FPC], F32, kind="ExternalInput")
    bv_ext = nc.dram_tensor("bv", [FPC], F32, kind="ExternalInput")
    bp_ext = nc.dram_tensor("bp", [E], BF16, kind="ExternalInput")
    out_ext = nc.dram_tensor("out", [TC, E], F32, kind="ExternalOutput")

    # A2A bounce buffers: group gi moves, for each dest core j and each of
    # its slots s, my (normalized) oT columns for tokens [512s + 64j, +64).
    o_loc = [
        nc.dram_tensor(f"o_loc{gi}", [NC, FPC, ATOK * len(g)], BF16)
        for gi, g in enumerate(A2A_GROUPS)
    ]
    o_gat = [
        nc.dram_tensor(f"o_gat{gi}", [NC, FPC, ATOK * len(g)], BF16)
        for gi, g in enumerate(A2A_GROUPS)
    ]

    with tile.TileContext(nc) as tc:
        with (
            tc.tile_pool(name="const", bufs=1) as cpool,
            tc.tile_pool(name="wqkv", bufs=1) as wpool,
            tc.tile_pool(name="persist", bufs=1) as apool,
            tc.tile_pool(name="xst", bufs=3) as xpool,
            tc.tile_pool(name="vT", bufs=2) as vtpool,
            tc.tile_pool(name="pT", bufs=34) as ppool,
            tc.tile_pool(name="og", bufs=2) as ogpool,
            tc.tile_pool(name="nrm", bufs=4) as npool,
            tc.tile_pool(name="ysb", bufs=2) as ypool,
            tc.tile_pool(name="ps_s", bufs=2, space="PSUM") as ps_s_pool,
            tc.tile_pool(name="ps_o", bufs=2, space="PSUM") as ps_o_pool,
            tc.tile_pool(name="ps_m", bufs=2, space="PSUM") as ps_m_pool,
        ):
            ident_f = cpool.tile([128, 128], F32)
            make_identity(nc, ident_f[:])
            ident = cpool.tile([128, 128], BF16)
            nc.vector.tensor_copy(ident[:], ident_f[:])
            ones_r = cpool.tile([1, 128], BF16)
            nc.vector.memset(ones_r[:], 1.0)
            bq_sb = cpool.tile([128, 1], F32)
            bk_sb = cpool.tile([128, 1], F32)
            bv_sb = cpool.tile([128, 1], F32)
            bp_sb = cpool.tile([1, E], BF16)
            nc.sync.dma_start(out=bq_sb[:], in_=bq_ext.ap().rearrange("(p a) -> p a", p=FPC))
            nc.sync.dma_start(out=bk_sb[:], in_=bk_ext.ap().rearrange("(p a) -> p a", p=FPC))
            nc.sync.dma_start(out=bv_sb[:], in_=bv_ext.ap().rearrange("(p a) -> p a", p=FPC))
            nc.sync.dma_start(out=bp_sb[:], in_=bp_ext.ap().rearrange("(a f) -> a f", a=1))

            wq_sb = wpool.tile([128, NEC, FPC], BF16)
            wk_sb = wpool.tile([128, NEC, FPC], BF16)
            wv_sb = wpool.tile([128, NEC, FPC], BF16)
            nc.sync.dma_start(out=wq_sb[:], in_=wq_ext.ap().rearrange("(j p) f -> p j f", p=128))
            nc.sync.dma_start(out=wk_sb[:], in_=wk_ext.ap().rearrange("(j p) f -> p j f", p=128))
            nc.sync.dma_start(out=wv_sb[:], in_=wv_ext.ap().rearrange("(j p) f -> p j f", p=128))
            wp_sb = apool.tile([128, NEC, E], BF16)

            qT = apool.tile([128, T], BF16)   # q features x all tokens
            kT = apool.tile([128, T], BF16)
            v_all = apool.tile([128, NTT, HPC, D + 1], BF16)  # [tok128, ktile, head, V|1]
            oT = apool.tile([128, T], BF16)   # attention out channels x tokens


            # ones column of v_all (softmax row-sum trick)
            nc.vector.memset(v_all[:, :, :, D : D + 1], 1.0)

            # ---------- phase building blocks ------------------------------
            def qkv_piece(st, piece, box):
                """One ~2us piece of the qkv projection for supertile st
                (0: x DMA + q, 1: k, 2: v, 3: vT transpose -> v_all).
                Sized so a mid-slot injection never starves the ACT."""
                if piece == 0:
                    x_t = xpool.tile([128, NEC, 512], BF16, tag="x")
                    nc.sync.dma_start(
                        out=x_t[:],
                        in_=xT_ext[:, st * 512 : (st + 1) * 512].rearrange(
                            "(j p) t -> p j t", p=128
                        ),
                    )
                    box["x"] = x_t
                    box["vt"] = vtpool.tile([128, 512], BF16, tag="vt", name="vT_st")
                if piece < 3:
                    w_sb, b_sb, dst = (
                        (wq_sb, bq_sb, qT[:, st * 512 : (st + 1) * 512]),
                        (wk_sb, bk_sb, kT[:, st * 512 : (st + 1) * 512]),
                        (wv_sb, bv_sb, box["vt"][:]),
                    )[piece]
                    ps = ps_m_pool.tile([128, 512], F32, tag="m")
                    for j in range(NEC):
                        nc.tensor.matmul(
                            ps[:],
                            w_sb[:, j, :],
                            box["x"][:, j, :],
                            start=(j == 0),
                            stop=(j == NEC - 1),
                        )
                    nc.vector.tensor_scalar_add(dst, ps[:], b_sb[:])
                else:
                    # V native layout via PE transpose of vT (transpose-mode
                    # output dtype must match input: bf16, half a PSUM bank)
                    ps_v = ps_m_pool.tile([128, 512], BF16, tag="m")
                    for i in range(4):
                        nc.tensor.transpose(
                            ps_v[:, 128 * i : 128 * (i + 1)],
                            box["vt"][:, 128 * i : 128 * (i + 1)],
                            ident[:],
                        )
                    nc.vector.tensor_copy(
                        v_all[:, st * 4 : (st + 1) * 4, :, 0:D],
                        ps_v[:].rearrange("p (i h d) -> p i h d", i=4, h=HPC),
                    )

            def qkv_supertile(st):
                box: dict = {}
                for piece in range(4):
                    qkv_piece(st, piece, box)

            def qk_group(s, h, tt, pts):
                """S^T tile (2 k-tiles) for slot s head h + exp -> P^T."""
                b = s // 4
                q0 = s * 512
                hp = 64 * h
                ps_s = ps_s_pool.tile([128, 1024], F32, tag="s")
                for i in range(2):
                    kti = b * KT_PER_B + tt * 2 + i
                    nc.tensor.matmul(
                        ps_s[:, 512 * i : 512 * (i + 1)],
                        kT[hp : hp + 64, 128 * kti : 128 * (kti + 1)],
                        qT[hp : hp + 64, q0 : q0 + 512],
                        start=True,
                        stop=True,
                        tile_position=(64 * h, 0),
                    )
                pt = ppool.tile([128, 1024], BF16, tag="p")
                nc.scalar.activation(pt[:], ps_s[:], AF.Exp, scale=0.125)
                pts[(h, tt)] = pt

            def pv_chunk(s, h, c, pts, ps_o_box):
                """4 accumulating PV matmuls (k-tiles 4c..4c+3) of slot s."""
                b = s // 4
                if c == 0:
                    ps_o_box[h] = ps_o_pool.tile([128, 512], F32, tag="o", name="ps_o")
                ps_o = ps_o_box[h]
                for kk in range(4):
                    kt = 4 * c + kk
                    kti = b * KT_PER_B + kt
                    nc.tensor.matmul(
                        ps_o[0 : D + 1, :],
                        v_all[:, kti, h, :],
                        pts[(h, kt // 2)][:, 512 * (kt % 2) : 512 * (kt % 2 + 1)],
                        start=(kt == 0),
                        stop=(kt == KT_PER_B - 1),
                    )

            def pv_evac(s, h, ps_o_box):
                """Normalize by the softmax denominator (row D) and evac."""
                q0 = s * 512
                hp = 64 * h
                ps_o = ps_o_box[h]
                # stage sums to SBUF: the custom-DVE recip mis-addresses a
                # PSUM source with nonzero base partition
                sm = npool.tile([1, 512], F32, tag="sm")
                nc.vector.tensor_copy(sm[:], ps_o[D : D + 1, :])
                rc = npool.tile([1, 512], F32, tag="rc")
                nc.vector.reciprocal_approx_fast(rc[:], sm[:])
                bcs = npool.tile([128, 512], F32, tag="bc")
                nc.gpsimd.partition_broadcast(bcs[:], rc[:])
                nc.vector.tensor_mul(
                    oT[hp : hp + 64, q0 : q0 + 512], ps_o[0:D, :], bcs[0:D, :]
                )

            def a2a_slot_done(s):
                """Stage slot s's oT chunks into its group's bounce buffer;
                fire the collective once the group's last slot is staged."""
                gi, idx = SLOT_GROUP[s]
                for j in range(NC):
                    c0 = s * 512 + ATOK * j
                    nc.sync.dma_start(
                        out=o_loc[gi][j, :, ATOK * idx : ATOK * (idx + 1)],
                        in_=oT[:, c0 : c0 + ATOK],
                    )
                if idx == len(A2A_GROUPS[gi]) - 1:
                    nc.gpsimd.collective_compute(
                        "AllToAll",
                        mybir.AluOpType.bypass,
                        replica_groups=[list(range(NC))],
                        ins=[o_loc[gi].ap().opt()],
                        outs=[o_gat[gi].ap().opt()],
                    )

            def og_load(s, og, half):
                """Gather slot s's A2A output (already normalized) into one
                half of a paired og tile (proj consumes 2 slots = 128 tok)."""
                gi, idx = SLOT_GROUP[s]
                for j in range(NC):
                    nc.sync.dma_start(
                        out=og[:, j, ATOK * half : ATOK * (half + 1)],
                        in_=o_gat[gi][j, :, ATOK * idx : ATOK * (idx + 1)],
                    )

            def proj_piece(m, og_n, cb):
                """c_proj output-column block cb for my 128 tokens of slot
                pair m (~2.4us of PE: fits a mid-slot injection)."""
                ps_y = ps_m_pool.tile([128, 512], F32, tag="m")
                for j in range(NEC):
                    nc.tensor.matmul(
                        ps_y[:],
                        og_n[:, j, :],
                        wp_sb[:, j, 512 * cb : 512 * (cb + 1)],
                        start=(j == 0),
                        stop=False,
                    )
                nc.tensor.matmul(
                    ps_y[:],
                    ones_r[:, 0:128],
                    bp_sb[:, 512 * cb : 512 * (cb + 1)],
                    start=False,
                    stop=True,
                )
                y_sb = ypool.tile([128, 512], F32, tag="y")
                nc.vector.tensor_copy(y_sb[:], ps_y[:])
                nc.sync.dma_start(
                    out=out_ext[128 * m : 128 * (m + 1), 512 * cb : 512 * (cb + 1)],
                    in_=y_sb[:],
                )

            def proj(m, og_n):
                for cb in range(2):
                    proj_piece(m, og_n, cb)

            # ---------- emission schedule ----------------------------------
            # Slot s computes QK+exp for q tokens [512s, 512s+512) while the
            # PE drains PV of slot s-1 between QK groups. qkv supertiles are
            # woven in: st0 up front, st1-3 inside slot 0 right before the
            # QK groups that consume them, st4-7 (batch 1) as filler in
            # slots 1-3. proj blocks inject after g==3 where the ACT has
            # backlog to hide the PE detour; A2A m fires as soon as its two
            # source slots are evacuated.
            # Ramp: wq then st0's x feed the very first matmuls; the bulkier
            # wk/wv transfers queue behind them.
            st_boxes: dict = {st: {} for st in range(NST)}
            qkv_piece(0, 0, st_boxes[0])
            for piece in range(1, 4):
                qkv_piece(0, piece, st_boxes[0])

            # filler schedule: supertile st fills slot st-3 (st4->slot1 ...
            # st7->slot4, each piece ~2us at g 1/3/5/end); slot 0 weaves
            # st1-3 as hard dependencies of its own QK groups.
            pts_prev = None
            ps_o_prev: dict = {}
            og_ns: dict = {}
            for s in range(NSLOT + 1):
                pts: dict = {}
                ps_o_box: dict = {}
                fill_st = s + 3 if 1 <= s <= 4 else None
                for g in range(8):
                    if s == 0 and g >= 2 and g % 2 == 0:
                        st = g // 2  # st 1,2,3: q/k parts gate QK below
                        qkv_piece(st, 0, st_boxes[st])
                        qkv_piece(st, 1, st_boxes[st])
                        if st >= 2:  # v parts of the previous supertile
                            qkv_piece(st - 1, 2, st_boxes[st - 1])
                            qkv_piece(st - 1, 3, st_boxes[st - 1])
                    if s < NSLOT:
                        for h in range(HPC):
                            qk_group(s, h, g, pts)
                    if s > 0:
                        h, c = (0, g) if g < 4 else (1, g - 4)
                        pv_chunk(s - 1, h, c, pts_prev, ps_o_prev)
                        if g == 3:
                            pv_evac(s - 1, 0, ps_o_prev)
                        elif g == 7:
                            pv_evac(s - 1, 1, ps_o_prev)
                            a2a_slot_done(s - 1)
                    # mid-slot fillers (~2us PE each: ACT backlog covers them)
                    if g in (1, 3, 5) and fill_st is not None:
                        qkv_piece(fill_st, g // 2, st_boxes[fill_st])
                    if g == 3 and s == 5:
                        proj_piece(0, og_ns[0], 0)
                    elif g == 5 and s == 5:
                        proj_piece(0, og_ns[0], 1)
                    elif g == 3 and s == 7:
                        proj_piece(1, og_ns[1], 0)
                    elif g == 5 and s == 7:
                        proj_piece(1, og_ns[1], 1)
                # end-of-slot work
                if fill_st is not None:
                    qkv_piece(fill_st, 3, st_boxes[fill_st])
                if s == 0:
                    qkv_piece(3, 2, st_boxes[3])
                    qkv_piece(3, 3, st_boxes[3])
                    nc.sync.dma_start(
                        out=wp_sb[:],
                        in_=wp_ext.ap().rearrange("(j p) f -> p j f", p=128),
                    )
                elif s == 4:
                    og_ns[0] = ogpool.tile([128, NC, 2 * ATOK], BF16, tag="og", name="og0")
                    og_load(0, og_ns[0], 0)
                    og_load(1, og_ns[0], 1)
                elif s == 6:
                    og_ns[1] = ogpool.tile([128, NC, 2 * ATOK], BF16, tag="og", name="og1")
                    og_load(2, og_ns[1], 0)
                    og_load(3, og_ns[1], 1)
                elif s == 7:
                    og_ns[2] = ogpool.tile([128, NC, 2 * ATOK], BF16, tag="og", name="og2")
                    og_load(4, og_ns[2], 0)
                    og_load(5, og_ns[2], 1)
                elif s == 8:
                    og_ns[3] = ogpool.tile([128, NC, 2 * ATOK], BF16, tag="og", name="og3")
                    og_load(6, og_ns[3], 0)
                    proj(2, og_ns[2])
                    og_load(7, og_ns[3], 1)
                    proj(3, og_ns[3])
                pts_prev = pts
                ps_o_prev = ps_o_box

    nc.compile()
    return nc


_NC_CACHE = None


def _get_nc():
    global _NC_CACHE
    if _NC_CACHE is None:
        _NC_CACHE = build_nc()
    return _NC_CACHE


def kernel(
    hidden_states: np.ndarray,
    c_attn_w: np.ndarray,
    c_attn_b: np.ndarray,
    c_proj_w: np.ndarray,
    c_proj_b: np.ndarray,
    _want_results_obj: bool = False,
    **_unused,
) -> np.ndarray:
    BF = ml_dtypes.bfloat16
    x = np.asarray(hidden_states, dtype=np.float32).reshape(T, E)
    xT = np.ascontiguousarray(x.T).astype(BF)
    w = np.asarray(c_attn_w, dtype=np.float32)
    battn = np.asarray(c_attn_b, dtype=np.float32)
    wp = np.ascontiguousarray(np.asarray(c_proj_w, dtype=np.float32)).astype(BF)
    bp = np.asarray(c_proj_b, dtype=np.float32).astype(BF)

    in_maps = []
    for c in range(NC):
        f0 = FPC * c
        in_maps.append(
            {
                "xT": xT,
                "wq": np.ascontiguousarray(w[:, f0 : f0 + FPC].astype(BF)),
                "wk": np.ascontiguousarray(w[:, E + f0 : E + f0 + FPC].astype(BF)),
                "wv": np.ascontiguousarray(w[:, 2 * E + f0 : 2 * E + f0 + FPC].astype(BF)),
                "wp": wp,
                "bq": np.ascontiguousarray(battn[f0 : f0 + FPC]),
                "bk": np.ascontiguousarray(battn[E + f0 : E + f0 + FPC]),
                "bv": np.ascontiguousarray(battn[2 * E + f0 : 2 * E + f0 + FPC]),
                "bp": bp,
            }
        )

    nc = _get_nc()
    res = run_bass_kernel_spmd(nc, in_maps, core_ids=list(range(NC)))
    # out row u of core c holds global token 512*(u//64) + 64*c + u%64
    y = np.empty((T, E), dtype=np.float32)
    for c in range(NC):
        for m in range(NSLOT):
            t0 = m * 512 + ATOK * c
            y[t0 : t0 + ATOK] = res.results[c]["out"][ATOK * m : ATOK * (m + 1)]
    out = y.reshape(B, S, E)
    if _want_results_obj:
        return out, res
    return out


# revision 47
# speedup vs baseline: 1.1521x; 1.0431x over previous
"""GPT-2 attention block (B=2, S=2048, E=1024, H=16) on 8 TRN2 NeuronCores.

Sharding: 8-way tensor parallel over heads (2 heads/core). Host passes
x pre-transposed (xT [E,T] bf16), so the qkv projection consumes it
directly with no on-device transposes. Four chunked AllToAlls reshard
attention output from head-sharded to token-sharded (1024 tokens each),
overlapping collectives with attention compute; each core computes the
c_proj output for its 4x128-token slices with full contraction.

All matmuls run in bf16 (full-rate PE + FWL weight loads); softmax
logits accumulate to bf16 PSUM tiles (1 bank each) so the S-tile pool
is 4 deep. Emission interleaves QK(s) with PV(s-1) and qkv/proj filler
so the PE stays dense (HAM warm) while ACT streams the exps.

Per-core dataflow:
  qT = Wq^T xT (+bq)  [128, 4096] bf16     (DVE evac, per-partition bias)
  kT, vT likewise; V native layout via PE transpose of vT
  per 512-token q tile: S^T = K Q^T (bf16 PSUM); P^T = exp(S^T/8) on ACT
  O'^T[65,512] = [V|1]^T P^T  (row 64 = softmax denominators, unnormalized)
  4x AllToAll of O'^T+sums (bf16) -> token-sharded og
  og_n = og * broadcast(1/sums)  (approx reciprocal, 16 lanes)
  y = og_n @ Wp + bp -> out [512, 1024] f32
"""

import sys

if "/opt/trn_rl_repo" not in sys.path:
    sys.path.insert(0, "/opt/trn_rl_repo")

import ml_dtypes
import numpy as np

import concourse.bass as bass  # noqa: F401
import concourse.mybir as mybir
from concourse import bacc, tile
from concourse.bass_utils import run_bass_kernel_spmd
from concourse.masks import make_identity

F32 = mybir.dt.float32
BF16 = mybir.dt.bfloat16
AF = mybir.ActivationFunctionType

B, S, E, H = 2, 2048, 1024, 16
D = E // H            # 64
NC = 8                # cores
HPC = H // NC         # 2 heads per core
FPC = HPC * D         # 128 per-core q/k/v feature count
T = B * S             # 4096 tokens, batch-major
TC = T // NC          # 512 output tokens per core
NTT = T // 128        # 32 token tiles of 128
NST = T // 512        # 8 token supertiles of 512
NEC = E // 128        # 8 contraction chunks
KT_PER_B = S // 128   # 16 k tiles per batch
NSLOT = T // 512      # 8 attention slots of 512 q tokens
ATOK = 64             # tokens per core per slot-chunk
# A2A groups of slots: early ones pair 2 slots (fewer collectives, the
# ~13us-floor cc stream must not backlog); the last two are single-slot
# so the tail collective after the final evac is minimal.
A2A_GROUPS = [(0, 1), (2, 3), (4, 5), (6,), (7,)]
SLOT_GROUP = {s: (gi, idx) for gi, g in enumerate(A2A_GROUPS) for idx, s in enumerate(g)}


def build_nc():
    nc = bacc.Bacc("TRN2", target_bir_lowering=False, debug=False, num_devices=NC)

    xT_ext = nc.dram_tensor("xT", [E, T], BF16, kind="ExternalInput")
    wq_ext = nc.dram_tensor("wq", [E, FPC], BF16, kind="ExternalInput")
    wk_ext = nc.dram_tensor("wk", [E, FPC], BF16, kind="ExternalInput")
    wv_ext = nc.dram_tensor("wv", [E, FPC], BF16, kind="ExternalInput")
    wp_ext = nc.dram_tensor("wp", [E, E], BF16, kind="ExternalInput")
    bq_ext = nc.dram_tensor("bq", [FPC], F32, kind="ExternalInput")
    bk_ext = nc.dram_tensor("bk", [FPC], F32, kind="ExternalInput")
    bv_ext = nc.dram_tensor("bv", [FPC], F32, kind="ExternalInput")
    bp_ext = nc.dram_tensor("bp", [E], BF16, kind="ExternalInput")
    out_ext = nc.dram_tensor("out", [TC, E], F32, kind="ExternalOutput")

    # A2A bounce buffers: group gi moves, for each dest core j and each of
    # its slots s, my (normalized) oT columns for tokens [512s + 64j, +64).
    o_loc = [
        nc.dram_tensor(f"o_loc{gi}", [NC, FPC, ATOK * len(g)], BF16)
        for gi, g in enumerate(A2A_GROUPS)
    ]
    o_gat = [
        nc.dram_tensor(f"o_gat{gi}", [NC, FPC, ATOK * len(g)], BF16)
        for gi, g in enumerate(A2A_GROUPS)
    ]

    with tile.TileContext(nc) as tc:
        with (
            tc.tile_pool(name="const", bufs=1) as cpool,
            tc.tile_pool(name="wqkv", bufs=1) as wpool,
            tc.tile_pool(name="persist", bufs=1) as apool,
            tc.tile_pool(name="xst", bufs=3) as xpool,
            tc.tile_pool(name="vT", bufs=2) as vtpool,
            tc.tile_pool(name="pT", bufs=34) as ppool,
            tc.tile_pool(name="og", bufs=2) as ogpool,
            tc.tile_pool(name="nrm", bufs=4) as npool,
            tc.tile_pool(name="ysb", bufs=2) as ypool,
            tc.tile_pool(name="ps_s", bufs=2, space="PSUM") as ps_s_pool,
            tc.tile_pool(name="ps_o", bufs=2, space="PSUM") as ps_o_pool,
            tc.tile_pool(name="ps_m", bufs=2, space="PSUM") as ps_m_pool,
        ):
            ident_f = cpool.tile([128, 128], F32)
            make_identity(nc, ident_f[:])
            ident = cpool.tile([128, 128], BF16)
            nc.vector.tensor_copy(ident[:], ident_f[:])
            ones_r = cpool.tile([1, 128], BF16)
            nc.vector.memset(ones_r[:], 1.0)
            bq_sb = cpool.tile([128, 1], F32)
            bk_sb = cpool.tile([128, 1], F32)
            bv_sb = cpool.tile([128, 1], F32)
            bp_sb = cpool.tile([1, E], BF16)
            nc.sync.dma_start(out=bq_sb[:], in_=bq_ext.ap().rearrange("(p a) -> p a", p=FPC))
            nc.sync.dma_start(out=bk_sb[:], in_=bk_ext.ap().rearrange("(p a) -> p a", p=FPC))
            nc.sync.dma_start(out=bv_sb[:], in_=bv_ext.ap().rearrange("(p a) -> p a", p=FPC))
            nc.sync.dma_start(out=bp_sb[:], in_=bp_ext.ap().rearrange("(a f) -> a f", a=1))

            wq_sb = wpool.tile([128, NEC, FPC], BF16)
            wk_sb = wpool.tile([128, NEC, FPC], BF16)
            wv_sb = wpool.tile([128, NEC, FPC], BF16)
            nc.sync.dma_start(out=wq_sb[:], in_=wq_ext.ap().rearrange("(j p) f -> p j f", p=128))
            wp_sb = apool.tile([128, NEC, E], BF16)

            qT = apool.tile([128, T], BF16)   # q features x all tokens
            kT = apool.tile([128, T], BF16)
            v_all = apool.tile([128, NTT, HPC, D + 1], BF16)  # [tok128, ktile, head, V|1]
            oT = apool.tile([128, T], BF16)   # attention out channels x tokens


            # ones column of v_all (softmax row-sum trick)
            nc.vector.memset(v_all[:, :, :, D : D + 1], 1.0)

            # ---------- phase building blocks ------------------------------
            def qkv_piece(st, piece, box):
                """One ~2us piece of the qkv projection for supertile st
                (0: x DMA + q, 1: k, 2: v, 3: vT transpose -> v_all).
                Sized so a mid-slot injection never starves the ACT."""
                if piece == 0:
                    x_t = xpool.tile([128, NEC, 512], BF16, tag="x")
                    nc.sync.dma_start(
                        out=x_t[:],
                        in_=xT_ext[:, st * 512 : (st + 1) * 512].rearrange(
                            "(j p) t -> p j t", p=128
                        ),
                    )
                    box["x"] = x_t
                    box["vt"] = vtpool.tile([128, 512], BF16, tag="vt", name="vT_st")
                if piece < 3:
                    w_sb, b_sb, dst = (
                        (wq_sb, bq_sb, qT[:, st * 512 : (st + 1) * 512]),
                        (wk_sb, bk_sb, kT[:, st * 512 : (st + 1) * 512]),
                        (wv_sb, bv_sb, box["vt"][:]),
                    )[piece]
                    ps = ps_m_pool.tile([128, 512], F32, tag="m")
                    for j in range(NEC):
                        nc.tensor.matmul(
                            ps[:],
                            w_sb[:, j, :],
                            box["x"][:, j, :],
                            start=(j == 0),
                            stop=(j == NEC - 1),
                        )
                    nc.vector.tensor_scalar_add(dst, ps[:], b_sb[:])
                else:
                    # V native layout via PE transpose of vT (transpose-mode
                    # output dtype must match input: bf16, half a PSUM bank)
                    ps_v = ps_m_pool.tile([128, 512], BF16, tag="m")
                    for i in range(4):
                        nc.tensor.transpose(
                            ps_v[:, 128 * i : 128 * (i + 1)],
                            box["vt"][:, 128 * i : 128 * (i + 1)],
                            ident[:],
                        )
                    nc.vector.tensor_copy(
                        v_all[:, st * 4 : (st + 1) * 4, :, 0:D],
                        ps_v[:].rearrange("p (i h d) -> p i h d", i=4, h=HPC),
                    )

            def qkv_supertile(st):
                box: dict = {}
                for piece in range(4):
                    qkv_piece(st, piece, box)

            def qk_group(s, h, tt, pts):
                """S^T tile (2 k-tiles) for slot s head h + exp -> P^T."""
                b = s // 4
                q0 = s * 512
                hp = 64 * h
                ps_s = ps_s_pool.tile([128, 1024], F32, tag="s")
                for i in range(2):
                    kti = b * KT_PER_B + tt * 2 + i
                    nc.tensor.matmul(
                        ps_s[:, 512 * i : 512 * (i + 1)],
                        kT[hp : hp + 64, 128 * kti : 128 * (kti + 1)],
                        qT[hp : hp + 64, q0 : q0 + 512],
                        start=True,
                        stop=True,
                        tile_position=(64 * h, 0),
                    )
                pt = ppool.tile([128, 1024], BF16, tag="p")
                nc.scalar.activation(pt[:], ps_s[:], AF.Exp, scale=0.125)
                pts[(h, tt)] = pt

            def pv_chunk(s, h, c, pts, ps_o_box):
                """4 accumulating PV matmuls (k-tiles 4c..4c+3) of slot s."""
                b = s // 4
                if c == 0:
                    ps_o_box[h] = ps_o_pool.tile([128, 512], F32, tag="o", name="ps_o")
                ps_o = ps_o_box[h]
                for kk in range(4):
                    kt = 4 * c + kk
                    kti = b * KT_PER_B + kt
                    nc.tensor.matmul(
                        ps_o[0 : D + 1, :],
                        v_all[:, kti, h, :],
                        pts[(h, kt // 2)][:, 512 * (kt % 2) : 512 * (kt % 2 + 1)],
                        start=(kt == 0),
                        stop=(kt == KT_PER_B - 1),
                    )

            def pv_evac(s, h, ps_o_box):
                """Normalize by the softmax denominator (row D) and evac."""
                q0 = s * 512
                hp = 64 * h
                ps_o = ps_o_box[h]
                # stage sums to SBUF: the custom-DVE recip mis-addresses a
                # PSUM source with nonzero base partition
                sm = npool.tile([1, 512], F32, tag="sm")
                nc.vector.tensor_copy(sm[:], ps_o[D : D + 1, :])
                rc = npool.tile([1, 512], F32, tag="rc")
                nc.vector.reciprocal_approx_fast(rc[:], sm[:])
                bcs = npool.tile([128, 512], F32, tag="bc")
                nc.gpsimd.partition_broadcast(bcs[:], rc[:])
                nc.vector.tensor_mul(
                    oT[hp : hp + 64, q0 : q0 + 512], ps_o[0:D, :], bcs[0:D, :]
                )

            def a2a_slot_done(s):
                """Stage slot s's oT chunks into its group's bounce buffer
                (one strided DMA); fire the collective once the group's last
                slot is staged."""
                gi, idx = SLOT_GROUP[s]
                nc.sync.dma_start(
                    out=o_loc[gi]
                    .ap()[:, :, ATOK * idx : ATOK * (idx + 1)]
                    .rearrange("j p t -> p j t"),
                    in_=oT[:, s * 512 : (s + 1) * 512].rearrange(
                        "p (j t) -> p j t", j=NC
                    ),
                )
                if idx == len(A2A_GROUPS[gi]) - 1:
                    nc.gpsimd.collective_compute(
                        "AllToAll",
                        mybir.AluOpType.bypass,
                        replica_groups=[list(range(NC))],
                        ins=[o_loc[gi].ap().opt()],
                        outs=[o_gat[gi].ap().opt()],
                    )

            def og_load(s, og, half):
                """Gather slot s's A2A output (already normalized) into one
                half of a paired og tile (proj consumes 2 slots = 128 tok),
                as one strided DMA."""
                gi, idx = SLOT_GROUP[s]
                nc.sync.dma_start(
                    out=og[:, :, ATOK * half : ATOK * (half + 1)],
                    in_=o_gat[gi]
                    .ap()[:, :, ATOK * idx : ATOK * (idx + 1)]
                    .rearrange("j p t -> p j t"),
                )

            def proj_piece(m, og_n, cb):
                """c_proj output-column block cb for my 128 tokens of slot
                pair m (~2.4us of PE: fits a mid-slot injection)."""
                ps_y = ps_m_pool.tile([128, 512], F32, tag="m")
                for j in range(NEC):
                    nc.tensor.matmul(
                        ps_y[:],
                        og_n[:, j, :],
                        wp_sb[:, j, 512 * cb : 512 * (cb + 1)],
                        start=(j == 0),
                        stop=False,
                    )
                nc.tensor.matmul(
                    ps_y[:],
                    ones_r[:, 0:128],
                    bp_sb[:, 512 * cb : 512 * (cb + 1)],
                    start=False,
                    stop=True,
                )
                y_sb = ypool.tile([128, 512], F32, tag="y")
                nc.vector.tensor_copy(y_sb[:], ps_y[:])
                nc.sync.dma_start(
                    out=out_ext[128 * m : 128 * (m + 1), 512 * cb : 512 * (cb + 1)],
                    in_=y_sb[:],
                )

            def proj(m, og_n):
                for cb in range(2):
                    proj_piece(m, og_n, cb)

            # ---------- emission schedule ----------------------------------
            # Slot s computes QK+exp for q tokens [512s, 512s+512) while the
            # PE drains PV of slot s-1 between QK groups. qkv supertiles are
            # woven in: st0 up front, st1-3 inside slot 0 right before the
            # QK groups that consume them, st4-7 (batch 1) as filler in
            # slots 1-3. proj blocks inject after g==3 where the ACT has
            # backlog to hide the PE detour; A2A m fires as soon as its two
            # source slots are evacuated.
            # Ramp: wq then st0's x feed the very first matmuls; the bulkier
            # wk/wv transfers queue behind them. st0's v pieces defer into
            # slot 0 so the first QK groups (and exps) start sooner.
            st_boxes: dict = {st: {} for st in range(NST)}
            qkv_piece(0, 0, st_boxes[0])
            nc.sync.dma_start(out=wk_sb[:], in_=wk_ext.ap().rearrange("(j p) f -> p j f", p=128))
            nc.sync.dma_start(out=wv_sb[:], in_=wv_ext.ap().rearrange("(j p) f -> p j f", p=128))
            qkv_piece(0, 1, st_boxes[0])

            # filler schedule: supertile st fills slot st-3 (st4->slot1 ...
            # st7->slot4, each piece ~2us at g 1/3/5/end); slot 0 weaves
            # st1-3 as hard dependencies of its own QK groups.
            pts_prev = None
            ps_o_prev: dict = {}
            og_ns: dict = {}
            for s in range(NSLOT + 1):
                pts: dict = {}
                ps_o_box: dict = {}
                fill_st = s + 3 if 1 <= s <= 4 else None
                for g in range(8):
                    if s == 0 and g in (1, 2) :
                        qkv_piece(0, g + 1, st_boxes[0])  # st0 v parts
                    if s == 0 and g >= 2 and g % 2 == 0:
                        st = g // 2  # st 1,2,3: q/k parts gate QK below
                        qkv_piece(st, 0, st_boxes[st])
                        qkv_piece(st, 1, st_boxes[st])
                        if st >= 2:  # v parts of the previous supertile
                            qkv_piece(st - 1, 2, st_boxes[st - 1])
                            qkv_piece(st - 1, 3, st_boxes[st - 1])
                    if s < NSLOT:
                        for h in range(HPC):
                            qk_group(s, h, g, pts)
                    if s > 0:
                        h, c = (0, g) if g < 4 else (1, g - 4)
                        pv_chunk(s - 1, h, c, pts_prev, ps_o_prev)
                        if g == 3:
                            pv_evac(s - 1, 0, ps_o_prev)
                        elif g == 7:
                            pv_evac(s - 1, 1, ps_o_prev)
                            a2a_slot_done(s - 1)
                    # mid-slot fillers (~2us PE each: ACT backlog covers them)
                    if g in (1, 3, 5) and fill_st is not None:
                        qkv_piece(fill_st, g // 2, st_boxes[fill_st])
                    if g == 1 and s == 7:
                        og_ns[2] = ogpool.tile(
                            [128, NC, 2 * ATOK], BF16, tag="og", name="og2"
                        )
                        og_load(4, og_ns[2], 0)
                        og_load(5, og_ns[2], 1)
                    if g == 3 and s == 5:
                        proj_piece(0, og_ns[0], 0)
                    elif g == 5 and s == 5:
                        proj_piece(0, og_ns[0], 1)
                    elif g == 3 and s == 7:
                        proj_piece(1, og_ns[1], 0)
                    elif g == 5 and s == 7:
                        proj_piece(1, og_ns[1], 1)
                # end-of-slot work
                if fill_st is not None:
                    qkv_piece(fill_st, 3, st_boxes[fill_st])
                if s == 0:
                    qkv_piece(3, 2, st_boxes[3])
                    qkv_piece(3, 3, st_boxes[3])
                    nc.sync.dma_start(
                        out=wp_sb[:],
                        in_=wp_ext.ap().rearrange("(j p) f -> p j f", p=128),
                    )
                elif s == 4:
                    og_ns[0] = ogpool.tile([128, NC, 2 * ATOK], BF16, tag="og", name="og0")
                    og_load(0, og_ns[0], 0)
                    og_load(1, og_ns[0], 1)
                elif s == 6:
                    og_ns[1] = ogpool.tile([128, NC, 2 * ATOK], BF16, tag="og", name="og1")
                    og_load(2, og_ns[1], 0)
                    og_load(3, og_ns[1], 1)
                elif s == 8:
                    og_ns[3] = ogpool.tile([128, NC, 2 * ATOK], BF16, tag="og", name="og3")
                    og_load(6, og_ns[3], 0)
                    proj(2, og_ns[2])
                    og_load(7, og_ns[3], 1)
                    proj(3, og_ns[3])
                pts_prev = pts
                ps_o_prev = ps_o_box

    nc.compile()
    return nc


_NC_CACHE = None


def _get_nc():
    global _NC_CACHE
    if _NC_CACHE is None:
        _NC_CACHE = build_nc()
    return _NC_CACHE


def kernel(
    hidden_states: np.ndarray,
    c_attn_w: np.ndarray,
    c_attn_b: np.ndarray,
    c_proj_w: np.ndarray,
    c_proj_b: np.ndarray,
    _want_results_obj: bool = False,
    **_unused,
) -> np.ndarray:
    BF = ml_dtypes.bfloat16
    x = np.asarray(hidden_states, dtype=np.float32).reshape(T, E)
    xT = np.ascontiguousarray(x.T).astype(BF)
    w = np.asarray(c_attn_w, dtype=np.float32)
    battn = np.asarray(c_attn_b, dtype=np.float32)
    wp = np.ascontiguousarray(np.asarray(c_proj_w, dtype=np.float32)).astype(BF)
    bp = np.asarray(c_proj_b, dtype=np.float32).astype(BF)

    in_maps = []
    for c in range(NC):
        f0 = FPC * c
        in_maps.append(
            {
                "xT": xT,
                "wq": np.ascontiguousarray(w[:, f0 : f0 + FPC].astype(BF)),
                "wk": np.ascontiguousarray(w[:, E + f0 : E + f0 + FPC].astype(BF)),
                "wv": np.ascontiguousarray(w[:, 2 * E + f0 : 2 * E + f0 + FPC].astype(BF)),
                "wp": wp,
                "bq": np.ascontiguousarray(battn[f0 : f0 + FPC]),
                "bk": np.ascontiguousarray(battn[E + f0 : E + f0 + FPC]),
                "bv": np.ascontiguousarray(battn[2 * E + f0 : 2 * E + f0 + FPC]),
                "bp": bp,
            }
        )

    nc = _get_nc()
    res = run_bass_kernel_spmd(nc, in_maps, core_ids=list(range(NC)))
    # out row u of core c holds global token 512*(u//64) + 64*c + u%64
    y = np.empty((T, E), dtype=np.float32)
    for c in range(NC):
        for m in range(NSLOT):
            t0 = m * 512 + ATOK * c
            y[t0 : t0 + ATOK] = res.results[c]["out"][ATOK * m : ATOK * (m + 1)]
    out = y.reshape(B, S, E)
    if _want_results_obj:
        return out, res
    return out
